# revision 12
# baseline (speedup 1.0000x reference)
"""MoE (group-limited top-k routing) Trainium2 kernel, expert-parallel on 8 cores.

Strategy:
  - Host (numpy): gate softmax + group-limited top-4 routing (control plane,
    ~0.06% of FLOPs), token dispatch (gather per expert) and final combine.
  - Device (8 NeuronCores, SPMD): core c owns experts 2c, 2c+1. Each expert's
    routed tokens (padded to an adaptive capacity) run the SwiGLU FFN in fp32r
    at full PE rate; the gate weight is fused into the down-proj epilogue.
    The shared expert is inter-dim sharded (2816/8=352, padded to 384 per
    core) and each core computes a partial z for all 2048 tokens; host sums
    the partials.
  - All device matmuls keep features on partitions and tokens on the moving
    free dim, so no transposes are needed anywhere on device. Host supplies
    every tensor pre-tiled in SBUF layout so all DMAs are contiguous.
  - Phase order: shared expert first (ws1/ws3 resident in SBUF, read once),
    then the two routed experts (weights streamed, read once).
"""

import numpy as np

# Model dims (hardcoded per problem spec nn_MoE_51616916963811)
D = 2048
INTER = 1408
E = 16
TOPK = 4
G = 4
TOPK_G = 2
T = 2048
SI = 2816           # shared inter dim
SI_SHARD = SI // 8  # 352
SI_PAD = 384        # padded to 3x128
ROUTE_SCALE = 1.0

NCORES = 8
ELOC = 2            # experts per core
TCHUNK = 512        # shared-expert token chunk
KD = D // 128       # 16 contraction chunks over D
KI = INTER // 128   # 11 tiles over INTER
KS = SI_PAD // 128  # 3 tiles over padded shared inter

CAP_MIN = 512       # capacity floor (expected count is exactly 512)

_CACHE = {}


def _pick_cap(max_count):
    """Round the max per-expert token count up to a multiple of 32.

    fp32r matmuls drop to 1/4 rate below a 256-wide moving dim, so chunks
    must stay >= 256: cap <= 512 is one chunk, else two halves."""
    cap = max(CAP_MIN, ((int(max_count) + 31) // 32) * 32)
    assert cap <= 1024
    if cap <= 512:
        return cap, (cap,)
    half = ((cap // 2) + 31) // 32 * 32
    return 2 * half, (half, half)


# ---------------------------------------------------------------- host gate --
def _route(x2d, Wg):
    """Replicates the reference gate in numpy float32.

    Returns topi [T, TOPK] int64 and weights [T, TOPK] float32."""
    logits = x2d.astype(np.float32) @ Wg.T.astype(np.float32)      # [T, E]
    m = logits.max(axis=-1, keepdims=True)
    ex = np.exp(logits - m)
    scores = ex / ex.sum(axis=-1, keepdims=True)                   # [T, E]
    sg = scores.reshape(T, G, E // G)
    gs = sg.max(axis=-1)                                           # [T, G]
    gidx = np.argsort(-gs, axis=1, kind="stable")[:, :TOPK_G]
    gmask = np.zeros((T, G), dtype=bool)
    np.put_along_axis(gmask, gidx, True, axis=1)
    masked = np.where(gmask[:, :, None], sg, -np.inf).reshape(T, E)
    topi = np.argsort(-masked, axis=1, kind="stable")[:, :TOPK]
    weights = np.take_along_axis(scores, topi, axis=1) * ROUTE_SCALE
    return topi, weights.astype(np.float32)


# ------------------------------------------------------------ host packing --
def _tile_kxm(w):
    """[R, C] weight -> lhsT tiles [R/128, 128(p), C/128 * 128] where
    tile[i, p, ko*128+m] = w[i*128+m, ko*128+p].  (w rows = output features,
    w cols = contraction dim.)"""
    R, C = w.shape
    ri, ci = R // 128, C // 128
    return np.ascontiguousarray(
        w.reshape(ri, 128, ci, 128).transpose(0, 3, 2, 1)
    ).reshape(ri, 128, ci * 128)


def _tile_xT(xrows, cap):
    """[n, D] activations -> [128(p), KD, cap] with xT[p, ko, c] = x[c, ko*128+p],
    zero-padded to cap tokens."""
    n = xrows.shape[0]
    out = np.zeros((128, KD, cap), dtype=np.float32)
    xt = xrows.T.reshape(KD, 128, n).transpose(1, 0, 2)  # [128, KD, n]
    out[:, :, :n] = xt
    return out


# ------------------------------------------------------------- bass kernel --
def _build_nc(cap, cchunks):
    import concourse.bass as bass
    import concourse.tile as tile
    from concourse import bacc, mybir

    f32 = mybir.dt.float32
    f32r = mybir.dt.float32r
    AF = mybir.ActivationFunctionType

    nc = bacc.Bacc("TRN2", target_bir_lowering=False, debug=False,
                   enable_asserts=False)

    # Inputs (per core). All pre-tiled on host; fp32r for matmul operands.
    xg = nc.dram_tensor("xg", [ELOC, 128, KD, cap], f32r, kind="ExternalInput").ap()
    gw = nc.dram_tensor("gw", [ELOC, 128, cap], f32, kind="ExternalInput").ap()
    w1 = nc.dram_tensor("w1", [ELOC, KI, 128, KD * 128], f32r, kind="ExternalInput").ap()
    w3 = nc.dram_tensor("w3", [ELOC, KI, 128, KD * 128], f32r, kind="ExternalInput").ap()
    w2 = nc.dram_tensor("w2", [ELOC, KD, 128, KI * 128], f32r, kind="ExternalInput").ap()
    xt = nc.dram_tensor("xt", [T // TCHUNK, 128, KD, TCHUNK], f32r, kind="ExternalInput").ap()
    ws1 = nc.dram_tensor("ws1", [KS, 128, KD * 128], f32r, kind="ExternalInput").ap()
    ws3 = nc.dram_tensor("ws3", [KS, 128, KD * 128], f32r, kind="ExternalInput").ap()
    ws2 = nc.dram_tensor("ws2", [KD, 128, KS * 128], f32r, kind="ExternalInput").ap()
    # Outputs
    yt = nc.dram_tensor("yt", [ELOC, KD, 128, cap], f32, kind="ExternalOutput").ap()
    zt = nc.dram_tensor("zt", [KD, 128, T], f32, kind="ExternalOutput").ap()

    ctile_off = []
    off = 0
    for w in cchunks:
        ctile_off.append((off, w))
        off += w
    NCT = T // TCHUNK

    with tile.TileContext(nc) as tc:
        # Long-lived pools (manual lifetime management so phases overlap).
        xs = tc.alloc_tile_pool(name="xs", bufs=2)
        astg = tc.alloc_tile_pool(name="astg", bufs=2)   # act+stage [128,512]
        pg12 = tc.alloc_tile_pool(name="pg12", bufs=2, space="PSUM")
        pg3 = tc.alloc_tile_pool(name="pg3", bufs=3, space="PSUM")
        gwp = tc.alloc_tile_pool(name="gwp", bufs=1)
        htp = tc.alloc_tile_pool(name="htp", bufs=1)
        wg3 = tc.alloc_tile_pool(name="wg3", bufs=2)
        wg12 = tc.alloc_tile_pool(name="wg12", bufs=2)

        # ---------------- routed experts, weights streamed ----------------
        def g12(s, xg_s, ht, w1t_pre=None, w3t_pre=None):
            first_mm = None
            for i in range(KI):
                if i == 0 and w1t_pre is not None:
                    w1t, w3t = w1t_pre, w3t_pre
                else:
                    w1t = wg12.tile([128, KD * 128], f32r, tag="w1t", name=f"w1t{s}_{i}")
                    nc.sync.dma_start(w1t[:], w1[s, i])
                    w3t = wg12.tile([128, KD * 128], f32r, tag="w3t", name=f"w3t{s}_{i}")
                    nc.sync.dma_start(w3t[:], w3[s, i])
                for (c0, cw) in ctile_off:
                    p1 = pg12.tile([128, cw], f32, tag="p1", name="p1")
                    p3 = pg12.tile([128, cw], f32, tag="p3", name="p3")
                    for ko in range(KD):
                        m = nc.tensor.matmul(
                            p1[:], w1t[:, ko * 128:(ko + 1) * 128],
                            xg_s[:, ko, c0:c0 + cw],
                            start=(ko == 0), stop=(ko == KD - 1))
                        if first_mm is None:
                            first_mm = m
                    for ko in range(KD):
                        nc.tensor.matmul(
                            p3[:], w3t[:, ko * 128:(ko + 1) * 128],
                            xg_s[:, ko, c0:c0 + cw],
                            start=(ko == 0), stop=(ko == KD - 1))
                    a1 = astg.tile([128, TCHUNK], f32, tag="astg", name="a1")
                    nc.scalar.activation(a1[:, :cw], p1[:], AF.Silu)
                    nc.vector.tensor_mul(ht[:, i, c0:c0 + cw], a1[:, :cw], p3[:])
            return first_mm

        def g3(s, ht, gw_s):
            last_mm = None
            for d in range(KD):
                w2t = wg3.tile([128, KI * 128], f32r, tag="w2t", name=f"w2t{s}_{d}")
                nc.sync.dma_start(w2t[:], w2[s, d])
                for (c0, cw) in ctile_off:
                    py = pg3.tile([128, cw], f32, tag="py", name="py")
                    for io in range(KI):
                        last_mm = nc.tensor.matmul(
                            py[:], w2t[:, io * 128:(io + 1) * 128],
                            ht[:, io, c0:c0 + cw],
                            start=(io == 0), stop=(io == KI - 1))
                    st = astg.tile([128, TCHUNK], f32, tag="astg", name="st")
                    nc.vector.tensor_mul(st[:, :cw], py[:], gw_s[:, c0:c0 + cw])
                    nc.sync.dma_start(yt[s, d, :, c0:c0 + cw], st[:, :cw])
            return last_mm

        # expert 0: weights for the first i-tile load before the bulky xg
        # so the first matmul group starts ~10us in
        w1t0 = wg12.tile([128, KD * 128], f32r, tag="w1t", name="w1t0_0")
        nc.sync.dma_start(w1t0[:], w1[0, 0])
        w3t0 = wg12.tile([128, KD * 128], f32r, tag="w3t", name="w3t0_0")
        nc.sync.dma_start(w3t0[:], w3[0, 0])

        xg0 = xs.tile([128, KD, cap], f32r, tag="x", name="xg0")
        for (c0, cw) in ctile_off:
            nc.sync.dma_start(xg0[:, :, c0:c0 + cw], xg[0, :, :, c0:c0 + cw])
        gw0 = gwp.tile([128, cap], f32, tag="gw", name="gw0")
        nc.sync.dma_start(gw0[:], gw[0])
        ht0 = htp.tile([128, KI, cap], f32r, tag="ht", name="ht0")
        g12(0, xg0, ht0, w1t_pre=w1t0, w3t_pre=w3t0)
        g3_last0 = g3(0, ht0, gw0)

        xg1 = xs.tile([128, KD, cap], f32r, tag="x", name="xg1")
        for (c0, cw) in ctile_off:
            nc.sync.dma_start(xg1[:, :, c0:c0 + cw], xg[1, :, :, c0:c0 + cw])
        gw1 = gwp.tile([128, cap], f32, tag="gw", name="gw1")
        nc.sync.dma_start(gw1[:], gw[1])
        ht1 = htp.tile([128, KI, cap], f32r, tag="ht", name="ht1")
        first_g12_e1 = g12(1, xg1, ht1)
        # ht single buffer: keep PE order G3(e0) -> G12(e1)
        tile.add_dep_helper(first_g12_e1.ins, g3_last0.ins, sync=False,
                            reason="ht single-buffer: G12(e1) after G3(e0)")

        # wg12 done: release so the shared-expert weights can load during
        # G3(e1) and shared S1 matmuls can fill its DMA-stall gaps
        wg12.release()
        wsr = tc.alloc_tile_pool(name="wsr", bufs=1)
        ws1r = wsr.tile([128, KS, KD * 128], f32r, tag="ws1r", name="ws1r")
        ws3r = wsr.tile([128, KS, KD * 128], f32r, tag="ws3r", name="ws3r")
        for i in range(KS):
            nc.sync.dma_start(ws1r[:, i], ws1[i])
            nc.sync.dma_start(ws3r[:, i], ws3[i])

        ws2rp = tc.alloc_tile_pool(name="ws2rp", bufs=1)
        ws2r = ws2rp.tile([128, KD, KS * 128], f32r, tag="ws2r", name="ws2r")
        for d in range(KD):
            nc.sync.dma_start(ws2r[:, d], ws2[d])

        g3_last1 = g3(1, ht1, gw1)

        # -------- shared expert (inter-sharded), weights resident, fused ----
        xt_tiles = {}
        xt_tiles[0] = xs.tile([128, KD, cap], f32r, tag="x", name="xt0")
        nc.sync.dma_start(xt_tiles[0][:, :, :TCHUNK], xt[0])
        s2_last_mm = None
        for ct in range(NCT):
            if ct + 1 < NCT:
                xt_tiles[ct + 1] = xs.tile([128, KD, cap], f32r, tag="x",
                                           name=f"xt{ct + 1}")
                nc.sync.dma_start(xt_tiles[ct + 1][:, :, :TCHUNK], xt[ct + 1])
            xt_c = xt_tiles.pop(ct)
            hst = htp.tile([128, KS, TCHUNK], f32r, tag="ht", name=f"hst{ct}")
            first_s1_mm = None
            for i in range(KS):
                p1 = pg12.tile([128, TCHUNK], f32, tag="p1", name="p1")
                p3 = pg12.tile([128, TCHUNK], f32, tag="p3", name="p3")
                for ko in range(KD):
                    m = nc.tensor.matmul(
                        p1[:], ws1r[:, i, ko * 128:(ko + 1) * 128],
                        xt_c[:, ko, :TCHUNK],
                        start=(ko == 0), stop=(ko == KD - 1))
                    if first_s1_mm is None:
                        first_s1_mm = m
                for ko in range(KD):
                    nc.tensor.matmul(
                        p3[:], ws3r[:, i, ko * 128:(ko + 1) * 128],
                        xt_c[:, ko, :TCHUNK],
                        start=(ko == 0), stop=(ko == KD - 1))
                a1 = astg.tile([128, TCHUNK], f32, tag="astg", name="a1")
                nc.scalar.activation(a1[:], p1[:], AF.Silu)
                nc.vector.tensor_mul(hst[:, i], a1[:], p3[:])

            # hst single buffer: keep PE order S2(ct-1) -> S1(ct)
            if s2_last_mm is not None:
                tile.add_dep_helper(
                    first_s1_mm.ins, s2_last_mm.ins, sync=False,
                    reason="hst single-buffer: S1(ct) after S2(ct-1)")

            for d in range(KD):
                py = pg3.tile([128, TCHUNK], f32, tag="py", name="py")
                for io in range(KS):
                    s2_last_mm = nc.tensor.matmul(
                        py[:], ws2r[:, d, io * 128:(io + 1) * 128],
                        hst[:, io],
                        start=(io == 0), stop=(io == KS - 1))
                st = astg.tile([128, TCHUNK], f32, tag="astg", name="st")
                # alternate the PSUM->SBUF drain between Scalar and Vector
                # engines so it keeps up with the 3-matmul fill
                if d % 2 == 0:
                    nc.scalar.activation(st[:], py[:], AF.Copy)
                else:
                    nc.vector.tensor_copy(st[:], py[:])
                nc.sync.dma_start(
                    zt[d, :, ct * TCHUNK:(ct + 1) * TCHUNK], st[:])

        ws2rp.release()
        wsr.release()
        wg3.release()
        htp.release()
        gwp.release()
        pg3.release()
        pg12.release()
        astg.release()
        xs.release()

    nc.compile()
    return nc


def _get_nc(cap, cchunks):
    key = ("nc", cap, cchunks)
    if key not in _CACHE:
        _CACHE[key] = _build_nc(cap, cchunks)
    return _CACHE[key]


# ------------------------------------------------------------------ kernel --
def kernel(x, Wg, W1, W3, W2, Ws1, Ws3, Ws2):
    from concourse.bass_utils import run_bass_kernel_spmd

    x = np.asarray(x, dtype=np.float32)
    x2d = np.ascontiguousarray(x.reshape(T, D))
    Wg = np.asarray(Wg, dtype=np.float32)
    W1 = np.asarray(W1, dtype=np.float32)
    W3 = np.asarray(W3, dtype=np.float32)
    W2 = np.asarray(W2, dtype=np.float32)
    Ws1 = np.asarray(Ws1, dtype=np.float32)
    Ws3 = np.asarray(Ws3, dtype=np.float32)
    Ws2 = np.asarray(Ws2, dtype=np.float32)

    # ---- host routing + dispatch ----
    topi, weights = _route(x2d, Wg)
    flat_e = topi.ravel()
    flat_t = np.repeat(np.arange(T), TOPK)
    flat_w = weights.ravel()
    order = np.argsort(flat_e, kind="stable")
    se, st_, sw = flat_e[order], flat_t[order], flat_w[order]
    bounds = np.searchsorted(se, np.arange(E + 1))
    tok_of = [st_[bounds[e]:bounds[e + 1]] for e in range(E)]
    wt_of = [sw[bounds[e]:bounds[e + 1]] for e in range(E)]

    cap, cchunks = _pick_cap(max(len(t) for t in tok_of))

    # ---- build per-core input maps ----
    xt_full = x2d.T.reshape(KD, 128, T).transpose(1, 0, 2)  # [128, KD, T]
    xt_tiles = np.ascontiguousarray(
        np.stack([xt_full[:, :, i * TCHUNK:(i + 1) * TCHUNK]
                  for i in range(T // TCHUNK)]))

    in_maps = []
    for c in range(NCORES):
        exps = [ELOC * c + s for s in range(ELOC)]
        xg_c = np.stack([_tile_xT(x2d[tok_of[e]], cap) for e in exps])
        gw_c = np.zeros((ELOC, 128, cap), dtype=np.float32)
        for s, e in enumerate(exps):
            gw_c[s, :, :len(wt_of[e])] = wt_of[e][None, :]
        w1_c = np.stack([_tile_kxm(W1[e]) for e in exps])
        w3_c = np.stack([_tile_kxm(W3[e]) for e in exps])
        w2_c = np.stack([_tile_kxm(W2[e]) for e in exps])

        lo = c * SI_SHARD
        ws1_s = np.zeros((SI_PAD, D), dtype=np.float32)
        ws1_s[:SI_SHARD] = Ws1[lo:lo + SI_SHARD]
        ws3_s = np.zeros((SI_PAD, D), dtype=np.float32)
        ws3_s[:SI_SHARD] = Ws3[lo:lo + SI_SHARD]
        ws2_s = np.zeros((D, SI_PAD), dtype=np.float32)
        ws2_s[:, :SI_SHARD] = Ws2[:, lo:lo + SI_SHARD]

        in_maps.append({
            "xg": xg_c, "gw": gw_c, "w1": w1_c, "w3": w3_c, "w2": w2_c,
            "xt": xt_tiles,
            "ws1": _tile_kxm(ws1_s), "ws3": _tile_kxm(ws3_s),
            "ws2": _tile_kxm(ws2_s),
        })

    # ---- run on 8 cores ----
    shapes = {
        "xg": (ELOC, 128, KD, cap), "gw": (ELOC, 128, cap),
        "w1": (ELOC, KI, 128, KD * 128), "w3": (ELOC, KI, 128, KD * 128),
        "w2": (ELOC, KD, 128, KI * 128),
        "xt": (T // TCHUNK, 128, KD, TCHUNK),
        "ws1": (KS, 128, KD * 128), "ws3": (KS, 128, KD * 128),
        "ws2": (KD, 128, KS * 128),
    }
    for m in in_maps:
        for k, v in m.items():
            assert v.shape == shapes[k], (k, v.shape, shapes[k])
            assert v.dtype == np.float32, (k, v.dtype)

    nc = _get_nc(cap, cchunks)
    res = run_bass_kernel_spmd(nc, in_maps, core_ids=list(range(NCORES)))
    _CACHE["last_results"] = res

    # ---- combine on host ----
    # routed: yt[s, d, p, c] = w * Y[c, d*128+p]
    cat_tok = []
    cat_rows = []
    for c in range(NCORES):
        ytc = res.results[c]["yt"]  # [ELOC, KD, 128, cap]
        for s in range(ELOC):
            e = ELOC * c + s
            n = len(tok_of[e])
            rows = ytc[s].reshape(D, cap).T[:n]  # [n, D]
            cat_tok.append(tok_of[e])
            cat_rows.append(rows)
    cat_tok = np.concatenate(cat_tok)
    cat_rows = np.concatenate(cat_rows, axis=0)
    order = np.argsort(cat_tok, kind="stable")
    y = cat_rows[order].reshape(T, TOPK, D).sum(axis=1)

    # shared: sum partials, zt[d, p, t] = Z[t, d*128+p]
    z_acc = res.results[0]["zt"].astype(np.float32).copy()
    for c in range(1, NCORES):
        z_acc += res.results[c]["zt"]
    z = z_acc.reshape(D, T).T  # [T, D]

    return (y + z).reshape(1, T, D).astype(np.float32)


# revision 13
# speedup vs baseline: 1.0606x; 1.0606x over previous
"""MoE (group-limited top-k routing) Trainium2 kernel, expert-parallel on 8 cores.

Strategy:
  - Host (numpy): gate softmax + group-limited top-4 routing (control plane,
    ~0.06% of FLOPs), token dispatch (gather per expert) and final combine.
  - Device (8 NeuronCores, SPMD): core c owns experts 2c, 2c+1. Each expert's
    routed tokens (padded to an adaptive capacity) run the SwiGLU FFN in fp32r
    at full PE rate; the gate weight is fused into the down-proj epilogue.
    The shared expert is inter-dim sharded (2816/8=352, padded to 384 per
    core) and each core computes a partial z for all 2048 tokens; host sums
    the partials.
  - All device matmuls keep features on partitions and tokens on the moving
    free dim, so no transposes are needed anywhere on device. Host supplies
    every tensor pre-tiled in SBUF layout so all DMAs are contiguous.
  - Phase order: shared expert first (ws1/ws3 resident in SBUF, read once),
    then the two routed experts (weights streamed, read once).
"""

import numpy as np

# Model dims (hardcoded per problem spec nn_MoE_51616916963811)
D = 2048
INTER = 1408
E = 16
TOPK = 4
G = 4
TOPK_G = 2
T = 2048
SI = 2816           # shared inter dim
SI_SHARD = SI // 8  # 352
SI_PAD = 384        # padded to 3x128
ROUTE_SCALE = 1.0

NCORES = 8
ELOC = 2            # experts per core
TCHUNK = 512        # shared-expert token chunk
KD = D // 128       # 16 contraction chunks over D
KI = INTER // 128   # 11 tiles over INTER
KS = SI_PAD // 128  # 3 tiles over padded shared inter

CAP_MIN = 512       # capacity floor (expected count is exactly 512)

_CACHE = {}


def _pick_cap(max_count):
    """Round the max per-expert token count up to a multiple of 32.

    fp32r matmuls drop to 1/4 rate below a 256-wide moving dim, so chunks
    must stay >= 256: cap <= 512 is one chunk, else two halves."""
    cap = max(CAP_MIN, ((int(max_count) + 31) // 32) * 32)
    assert cap <= 1024
    if cap <= 512:
        return cap, (cap,)
    half = ((cap // 2) + 31) // 32 * 32
    return 2 * half, (half, half)


# ---------------------------------------------------------------- host gate --
def _route(x2d, Wg):
    """Replicates the reference gate in numpy float32.

    Returns topi [T, TOPK] int64 and weights [T, TOPK] float32."""
    logits = x2d.astype(np.float32) @ Wg.T.astype(np.float32)      # [T, E]
    m = logits.max(axis=-1, keepdims=True)
    ex = np.exp(logits - m)
    scores = ex / ex.sum(axis=-1, keepdims=True)                   # [T, E]
    sg = scores.reshape(T, G, E // G)
    gs = sg.max(axis=-1)                                           # [T, G]
    gidx = np.argsort(-gs, axis=1, kind="stable")[:, :TOPK_G]
    gmask = np.zeros((T, G), dtype=bool)
    np.put_along_axis(gmask, gidx, True, axis=1)
    masked = np.where(gmask[:, :, None], sg, -np.inf).reshape(T, E)
    topi = np.argsort(-masked, axis=1, kind="stable")[:, :TOPK]
    weights = np.take_along_axis(scores, topi, axis=1) * ROUTE_SCALE
    return topi, weights.astype(np.float32)


# ------------------------------------------------------------ host packing --
def _tile_kxm(w):
    """[R, C] weight -> lhsT tiles [R/128, 128(p), C/128 * 128] where
    tile[i, p, ko*128+m] = w[i*128+m, ko*128+p].  (w rows = output features,
    w cols = contraction dim.)"""
    R, C = w.shape
    ri, ci = R // 128, C // 128
    return np.ascontiguousarray(
        w.reshape(ri, 128, ci, 128).transpose(0, 3, 2, 1)
    ).reshape(ri, 128, ci * 128)


def _tile_xT(xrows, cap):
    """[n, D] activations -> [128(p), KD, cap] with xT[p, ko, c] = x[c, ko*128+p],
    zero-padded to cap tokens."""
    n = xrows.shape[0]
    out = np.zeros((128, KD, cap), dtype=np.float32)
    xt = xrows.T.reshape(KD, 128, n).transpose(1, 0, 2)  # [128, KD, n]
    out[:, :, :n] = xt
    return out


# ------------------------------------------------------------- bass kernel --
def _build_nc(cap, cchunks):
    import concourse.bass as bass
    import concourse.tile as tile
    from concourse import bacc, mybir

    f32 = mybir.dt.float32
    f32r = mybir.dt.float32r
    AF = mybir.ActivationFunctionType

    nc = bacc.Bacc("TRN2", target_bir_lowering=False, debug=False,
                   enable_asserts=False)

    # Inputs (per core). All pre-tiled on host; fp32r for matmul operands.
    xg = nc.dram_tensor("xg", [ELOC, 128, KD, cap], f32r, kind="ExternalInput").ap()
    gw = nc.dram_tensor("gw", [ELOC, 128, cap], f32, kind="ExternalInput").ap()
    w1 = nc.dram_tensor("w1", [ELOC, KI, 128, KD * 128], f32r, kind="ExternalInput").ap()
    w3 = nc.dram_tensor("w3", [ELOC, KI, 128, KD * 128], f32r, kind="ExternalInput").ap()
    w2 = nc.dram_tensor("w2", [ELOC, KD, 128, KI * 128], f32r, kind="ExternalInput").ap()
    xt = nc.dram_tensor("xt", [T // TCHUNK, 128, KD, TCHUNK], f32r, kind="ExternalInput").ap()
    ws1 = nc.dram_tensor("ws1", [KS, 128, KD * 128], f32r, kind="ExternalInput").ap()
    ws3 = nc.dram_tensor("ws3", [KS, 128, KD * 128], f32r, kind="ExternalInput").ap()
    ws2 = nc.dram_tensor("ws2", [KD, 128, KS * 128], f32r, kind="ExternalInput").ap()
    # Outputs
    yt = nc.dram_tensor("yt", [ELOC, KD, 128, cap], f32, kind="ExternalOutput").ap()
    zt = nc.dram_tensor("zt", [KD, 128, T], f32, kind="ExternalOutput").ap()

    ctile_off = []
    off = 0
    for w in cchunks:
        ctile_off.append((off, w))
        off += w
    NCT = T // TCHUNK

    with tile.TileContext(nc) as tc:
        # ---------------- routed experts, weights streamed ----------------
        with tc.tile_pool(name="wg12", bufs=2) as wg12, \
             tc.tile_pool(name="wg3", bufs=3) as wg3, \
             tc.tile_pool(name="xs", bufs=2) as xs, \
             tc.tile_pool(name="htp", bufs=2) as htp, \
             tc.tile_pool(name="gwp", bufs=2) as gwp, \
             tc.tile_pool(name="actp", bufs=3) as actp, \
             tc.tile_pool(name="stg", bufs=3) as stg, \
             tc.tile_pool(name="pg12", bufs=2, space="PSUM") as pg12, \
             tc.tile_pool(name="pg3", bufs=3, space="PSUM") as pg3:

            for s in range(ELOC):
                if s == 0:
                    # first i-tile's weights load before the bulky xg so the
                    # first matmul group starts as early as possible
                    w1t0 = wg12.tile([128, KD * 128], f32r, tag="w1t", name="w1t0_0")
                    nc.sync.dma_start(w1t0[:], w1[0, 0])
                    w3t0 = wg12.tile([128, KD * 128], f32r, tag="w3t", name="w3t0_0")
                    nc.sync.dma_start(w3t0[:], w3[0, 0])
                xg_s = xs.tile([128, KD, cap], f32r, tag="x", name=f"xg{s}")
                for (c0, cw) in ctile_off:
                    nc.sync.dma_start(xg_s[:, :, c0:c0 + cw], xg[s, :, :, c0:c0 + cw])
                gw_s = gwp.tile([128, cap], f32, tag="gw", name=f"gw{s}")
                nc.sync.dma_start(gw_s[:], gw[s])

                ht = htp.tile([128, KI, cap], f32r, tag="ht", name=f"ht{s}")

                # GEMM1/2: hT[i, c] = silu(x @ W1^T) * (x @ W3^T), transposed
                for i in range(KI):
                    if s == 0 and i == 0:
                        w1t, w3t = w1t0, w3t0
                    else:
                        w1t = wg12.tile([128, KD * 128], f32r, tag="w1t", name=f"w1t{s}_{i}")
                        nc.sync.dma_start(w1t[:], w1[s, i])
                        w3t = wg12.tile([128, KD * 128], f32r, tag="w3t", name=f"w3t{s}_{i}")
                        nc.sync.dma_start(w3t[:], w3[s, i])
                    for (c0, cw) in ctile_off:
                        p1 = pg12.tile([128, cw], f32, tag="p1", name="p1")
                        p3 = pg12.tile([128, cw], f32, tag="p3", name="p3")
                        for ko in range(KD):
                            nc.tensor.matmul(
                                p1[:], w1t[:, ko * 128:(ko + 1) * 128],
                                xg_s[:, ko, c0:c0 + cw],
                                start=(ko == 0), stop=(ko == KD - 1))
                        for ko in range(KD):
                            nc.tensor.matmul(
                                p3[:], w3t[:, ko * 128:(ko + 1) * 128],
                                xg_s[:, ko, c0:c0 + cw],
                                start=(ko == 0), stop=(ko == KD - 1))
                        a1 = actp.tile([128, max(cchunks)], f32, tag="act", name="a1")
                        nc.scalar.activation(a1[:, :cw], p1[:], AF.Silu)
                        nc.vector.tensor_mul(ht[:, i, c0:c0 + cw], a1[:, :cw], p3[:])

                # GEMM3: yT[d, c] = (hT^T @ W2^T)^T * gate_weight
                for d in range(KD):
                    w2t = wg3.tile([128, KI * 128], f32r, tag="w2t", name=f"w2t{s}_{d}")
                    nc.sync.dma_start(w2t[:], w2[s, d])
                    for (c0, cw) in ctile_off:
                        py = pg3.tile([128, cw], f32, tag="py", name="py")
                        for io in range(KI):
                            nc.tensor.matmul(
                                py[:], w2t[:, io * 128:(io + 1) * 128],
                                ht[:, io, c0:c0 + cw],
                                start=(io == 0), stop=(io == KI - 1))
                        st = stg.tile([128, max(cchunks)], f32, tag="st", name="st")
                        nc.vector.tensor_mul(st[:, :cw], py[:], gw_s[:, c0:c0 + cw])
                        nc.sync.dma_start(yt[s, d, :, c0:c0 + cw], st[:, :cw])

        # -------- shared expert (inter-sharded), weights resident, fused ----
        with tc.tile_pool(name="wsr", bufs=1) as wsr, \
             tc.tile_pool(name="xss", bufs=2) as xss, \
             tc.tile_pool(name="hstp", bufs=2) as hstp, \
             tc.tile_pool(name="acts", bufs=3) as acts, \
             tc.tile_pool(name="stgs", bufs=3) as stgs, \
             tc.tile_pool(name="pg12s", bufs=2, space="PSUM") as pg12s, \
             tc.tile_pool(name="pg3s", bufs=3, space="PSUM") as pg3s:

            ws1r = wsr.tile([128, KS, KD * 128], f32r, tag="ws1r", name="ws1r")
            ws3r = wsr.tile([128, KS, KD * 128], f32r, tag="ws3r", name="ws3r")
            for i in range(KS):
                nc.sync.dma_start(ws1r[:, i], ws1[i])
                nc.sync.dma_start(ws3r[:, i], ws3[i])
            ws2r = wsr.tile([128, KD, KS * 128], f32r, tag="ws2r", name="ws2r")
            for d in range(KD):
                nc.sync.dma_start(ws2r[:, d], ws2[d])

            xt_tiles = {0: xss.tile([128, KD, TCHUNK], f32r, tag="xts", name="xt0")}
            nc.sync.dma_start(xt_tiles[0][:], xt[0])
            for ct in range(NCT):
                if ct + 1 < NCT:
                    xt_tiles[ct + 1] = xss.tile([128, KD, TCHUNK], f32r,
                                                tag="xts", name=f"xt{ct + 1}")
                    nc.sync.dma_start(xt_tiles[ct + 1][:], xt[ct + 1])
                xt_c = xt_tiles.pop(ct)
                hst = hstp.tile([128, KS, TCHUNK], f32r, tag="hst", name=f"hst{ct}")
                for i in range(KS):
                    p1 = pg12s.tile([128, TCHUNK], f32, tag="p1", name="p1")
                    p3 = pg12s.tile([128, TCHUNK], f32, tag="p3", name="p3")
                    for ko in range(KD):
                        nc.tensor.matmul(
                            p1[:], ws1r[:, i, ko * 128:(ko + 1) * 128],
                            xt_c[:, ko],
                            start=(ko == 0), stop=(ko == KD - 1))
                    for ko in range(KD):
                        nc.tensor.matmul(
                            p3[:], ws3r[:, i, ko * 128:(ko + 1) * 128],
                            xt_c[:, ko],
                            start=(ko == 0), stop=(ko == KD - 1))
                    a1 = acts.tile([128, TCHUNK], f32, tag="acts", name="a1")
                    nc.scalar.activation(a1[:], p1[:], AF.Silu)
                    nc.vector.tensor_mul(hst[:, i], a1[:], p3[:])

                for d in range(KD):
                    py = pg3s.tile([128, TCHUNK], f32, tag="py", name="py")
                    for io in range(KS):
                        nc.tensor.matmul(
                            py[:], ws2r[:, d, io * 128:(io + 1) * 128],
                            hst[:, io],
                            start=(io == 0), stop=(io == KS - 1))
                    st = stgs.tile([128, TCHUNK], f32, tag="stgs", name="st")
                    # alternate the PSUM->SBUF drain across two engines so it
                    # keeps up with the 3-matmul fill
                    if d % 2 == 0:
                        nc.scalar.activation(st[:], py[:], AF.Copy)
                    else:
                        nc.vector.tensor_copy(st[:], py[:])
                    nc.sync.dma_start(
                        zt[d, :, ct * TCHUNK:(ct + 1) * TCHUNK], st[:])

    nc.compile()
    return nc


def _get_nc(cap, cchunks):
    key = ("nc", cap, cchunks)
    if key not in _CACHE:
        _CACHE[key] = _build_nc(cap, cchunks)
    return _CACHE[key]


# ------------------------------------------------------------------ kernel --
def kernel(x, Wg, W1, W3, W2, Ws1, Ws3, Ws2):
    from concourse.bass_utils import run_bass_kernel_spmd

    x = np.asarray(x, dtype=np.float32)
    x2d = np.ascontiguousarray(x.reshape(T, D))
    Wg = np.asarray(Wg, dtype=np.float32)
    W1 = np.asarray(W1, dtype=np.float32)
    W3 = np.asarray(W3, dtype=np.float32)
    W2 = np.asarray(W2, dtype=np.float32)
    Ws1 = np.asarray(Ws1, dtype=np.float32)
    Ws3 = np.asarray(Ws3, dtype=np.float32)
    Ws2 = np.asarray(Ws2, dtype=np.float32)

    # ---- host routing + dispatch ----
    topi, weights = _route(x2d, Wg)
    flat_e = topi.ravel()
    flat_t = np.repeat(np.arange(T), TOPK)
    flat_w = weights.ravel()
    order = np.argsort(flat_e, kind="stable")
    se, st_, sw = flat_e[order], flat_t[order], flat_w[order]
    bounds = np.searchsorted(se, np.arange(E + 1))
    tok_of = [st_[bounds[e]:bounds[e + 1]] for e in range(E)]
    wt_of = [sw[bounds[e]:bounds[e + 1]] for e in range(E)]

    cap, cchunks = _pick_cap(max(len(t) for t in tok_of))

    # ---- build per-core input maps ----
    xt_full = x2d.T.reshape(KD, 128, T).transpose(1, 0, 2)  # [128, KD, T]
    xt_tiles = np.ascontiguousarray(
        np.stack([xt_full[:, :, i * TCHUNK:(i + 1) * TCHUNK]
                  for i in range(T // TCHUNK)]))

    in_maps = []
    for c in range(NCORES):
        exps = [ELOC * c + s for s in range(ELOC)]
        xg_c = np.stack([_tile_xT(x2d[tok_of[e]], cap) for e in exps])
        gw_c = np.zeros((ELOC, 128, cap), dtype=np.float32)
        for s, e in enumerate(exps):
            gw_c[s, :, :len(wt_of[e])] = wt_of[e][None, :]
        w1_c = np.stack([_tile_kxm(W1[e]) for e in exps])
        w3_c = np.stack([_tile_kxm(W3[e]) for e in exps])
        w2_c = np.stack([_tile_kxm(W2[e]) for e in exps])

        lo = c * SI_SHARD
        ws1_s = np.zeros((SI_PAD, D), dtype=np.float32)
        ws1_s[:SI_SHARD] = Ws1[lo:lo + SI_SHARD]
        ws3_s = np.zeros((SI_PAD, D), dtype=np.float32)
        ws3_s[:SI_SHARD] = Ws3[lo:lo + SI_SHARD]
        ws2_s = np.zeros((D, SI_PAD), dtype=np.float32)
        ws2_s[:, :SI_SHARD] = Ws2[:, lo:lo + SI_SHARD]

        in_maps.append({
            "xg": xg_c, "gw": gw_c, "w1": w1_c, "w3": w3_c, "w2": w2_c,
            "xt": xt_tiles,
            "ws1": _tile_kxm(ws1_s), "ws3": _tile_kxm(ws3_s),
            "ws2": _tile_kxm(ws2_s),
        })

    # ---- run on 8 cores ----
    shapes = {
        "xg": (ELOC, 128, KD, cap), "gw": (ELOC, 128, cap),
        "w1": (ELOC, KI, 128, KD * 128), "w3": (ELOC, KI, 128, KD * 128),
        "w2": (ELOC, KD, 128, KI * 128),
        "xt": (T // TCHUNK, 128, KD, TCHUNK),
        "ws1": (KS, 128, KD * 128), "ws3": (KS, 128, KD * 128),
        "ws2": (KD, 128, KS * 128),
    }
    for m in in_maps:
        for k, v in m.items():
            assert v.shape == shapes[k], (k, v.shape, shapes[k])
            assert v.dtype == np.float32, (k, v.dtype)

    nc = _get_nc(cap, cchunks)
    res = run_bass_kernel_spmd(nc, in_maps, core_ids=list(range(NCORES)))
    _CACHE["last_results"] = res

    # ---- combine on host ----
    # routed: yt[s, d, p, c] = w * Y[c, d*128+p]
    cat_tok = []
    cat_rows = []
    for c in range(NCORES):
        ytc = res.results[c]["yt"]  # [ELOC, KD, 128, cap]
        for s in range(ELOC):
            e = ELOC * c + s
            n = len(tok_of[e])
            rows = ytc[s].reshape(D, cap).T[:n]  # [n, D]
            cat_tok.append(tok_of[e])
            cat_rows.append(rows)
    cat_tok = np.concatenate(cat_tok)
    cat_rows = np.concatenate(cat_rows, axis=0)
    order = np.argsort(cat_tok, kind="stable")
    y = cat_rows[order].reshape(T, TOPK, D).sum(axis=1)

    # shared: sum partials, zt[d, p, t] = Z[t, d*128+p]
    z_acc = res.results[0]["zt"].astype(np.float32).copy()
    for c in range(1, NCORES):
        z_acc += res.results[c]["zt"]
    z = z_acc.reshape(D, T).T  # [T, D]

    return (y + z).reshape(1, T, D).astype(np.float32)


# revision 14
# speedup vs baseline: 1.1410x; 1.0758x over previous
"""MoE (group-limited top-k routing) Trainium2 kernel, expert-parallel on 8 cores.

Strategy:
  - Host (numpy): gate softmax + group-limited top-4 routing (control plane,
    ~0.06% of FLOPs), token dispatch (gather per expert) and final combine.
  - Device (8 NeuronCores, SPMD): core c owns experts 2c, 2c+1. Each expert's
    routed tokens (padded to an adaptive capacity) run the SwiGLU FFN in fp32r
    at full PE rate; the gate weight is fused into the down-proj epilogue.
    The shared expert is inter-dim sharded (2816/8=352, padded to 384 per
    core) and each core computes a partial z for all 2048 tokens; host sums
    the partials.
  - All device matmuls keep features on partitions and tokens on the moving
    free dim, so no transposes are needed anywhere on device. Host supplies
    every tensor pre-tiled in SBUF layout so all DMAs are contiguous.
  - Phase order: shared expert first (ws1/ws3 resident in SBUF, read once),
    then the two routed experts (weights streamed, read once).
"""

import numpy as np

# Model dims (hardcoded per problem spec nn_MoE_51616916963811)
D = 2048
INTER = 1408
E = 16
TOPK = 4
G = 4
TOPK_G = 2
T = 2048
SI = 2816           # shared inter dim
SI_SHARD = SI // 8  # 352
SI_PAD = 384        # padded to 3x128
ROUTE_SCALE = 1.0

NCORES = 8
ELOC = 2            # experts per core
TCHUNK = 512        # shared-expert token chunk
KD = D // 128       # 16 contraction chunks over D
KI = INTER // 128   # 11 tiles over INTER
KS = SI_PAD // 128  # 3 tiles over padded shared inter

CAP_MIN = 512       # capacity floor (expected count is exactly 512)

_CACHE = {}


def _pick_cap(max_count):
    """Round the max per-expert token count up to a multiple of 32.

    fp32r matmuls drop to 1/4 rate below a 256-wide moving dim, so chunks
    must stay >= 256: cap <= 512 is one chunk, else two halves."""
    cap = max(CAP_MIN, ((int(max_count) + 31) // 32) * 32)
    assert cap <= 1024
    if cap <= 512:
        return cap, (cap,)
    half = ((cap // 2) + 31) // 32 * 32
    return 2 * half, (half, half)


# ---------------------------------------------------------------- host gate --
def _route(x2d, Wg):
    """Replicates the reference gate in numpy float32.

    Returns topi [T, TOPK] int64 and weights [T, TOPK] float32."""
    logits = x2d.astype(np.float32) @ Wg.T.astype(np.float32)      # [T, E]
    m = logits.max(axis=-1, keepdims=True)
    ex = np.exp(logits - m)
    scores = ex / ex.sum(axis=-1, keepdims=True)                   # [T, E]
    sg = scores.reshape(T, G, E // G)
    gs = sg.max(axis=-1)                                           # [T, G]
    gidx = np.argsort(-gs, axis=1, kind="stable")[:, :TOPK_G]
    gmask = np.zeros((T, G), dtype=bool)
    np.put_along_axis(gmask, gidx, True, axis=1)
    masked = np.where(gmask[:, :, None], sg, -np.inf).reshape(T, E)
    topi = np.argsort(-masked, axis=1, kind="stable")[:, :TOPK]
    weights = np.take_along_axis(scores, topi, axis=1) * ROUTE_SCALE
    return topi, weights.astype(np.float32)


# ------------------------------------------------------------ host packing --
def _tile_kxm(w):
    """[R, C] weight -> lhsT tiles [R/128, 128(p), C/128 * 128] where
    tile[i, p, ko*128+m] = w[i*128+m, ko*128+p].  (w rows = output features,
    w cols = contraction dim.)"""
    R, C = w.shape
    ri, ci = R // 128, C // 128
    return np.ascontiguousarray(
        w.reshape(ri, 128, ci, 128).transpose(0, 3, 2, 1)
    ).reshape(ri, 128, ci * 128)


def _tile_xT(xrows, cap):
    """[n, D] activations -> [128(p), KD, cap] with xT[p, ko, c] = x[c, ko*128+p],
    zero-padded to cap tokens."""
    n = xrows.shape[0]
    out = np.zeros((128, KD, cap), dtype=np.float32)
    xt = xrows.T.reshape(KD, 128, n).transpose(1, 0, 2)  # [128, KD, n]
    out[:, :, :n] = xt
    return out


# ------------------------------------------------------------- bass kernel --
def _build_nc(cap, cchunks):
    import concourse.bass as bass
    import concourse.tile as tile
    from concourse import bacc, mybir

    f32 = mybir.dt.float32
    f32r = mybir.dt.float32r
    AF = mybir.ActivationFunctionType

    nc = bacc.Bacc("TRN2", target_bir_lowering=False, debug=False,
                   enable_asserts=False)

    # Inputs (per core). All pre-tiled on host; fp32r for matmul operands.
    xg = nc.dram_tensor("xg", [ELOC, 128, KD, cap], f32r, kind="ExternalInput").ap()
    gw = nc.dram_tensor("gw", [ELOC, 128, cap], f32, kind="ExternalInput").ap()
    w1 = nc.dram_tensor("w1", [ELOC, KI, 128, KD * 128], f32r, kind="ExternalInput").ap()
    w3 = nc.dram_tensor("w3", [ELOC, KI, 128, KD * 128], f32r, kind="ExternalInput").ap()
    w2 = nc.dram_tensor("w2", [ELOC, KD, 128, KI * 128], f32r, kind="ExternalInput").ap()
    xt = nc.dram_tensor("xt", [T // TCHUNK, 128, KD, TCHUNK], f32r, kind="ExternalInput").ap()
    ws1 = nc.dram_tensor("ws1", [KS, 128, KD * 128], f32r, kind="ExternalInput").ap()
    ws3 = nc.dram_tensor("ws3", [KS, 128, KD * 128], f32r, kind="ExternalInput").ap()
    ws2 = nc.dram_tensor("ws2", [KD, 128, KS * 128], f32r, kind="ExternalInput").ap()
    # Outputs
    yt = nc.dram_tensor("yt", [ELOC, KD, 128, cap], f32, kind="ExternalOutput").ap()
    zt = nc.dram_tensor("zt", [KD, 128, T], f32, kind="ExternalOutput").ap()

    ctile_off = []
    off = 0
    for w in cchunks:
        ctile_off.append((off, w))
        off += w

    with tile.TileContext(nc) as tc:
        # ---------------- routed experts, weights streamed ----------------
        with tc.tile_pool(name="wg12", bufs=3) as wg12, \
             tc.tile_pool(name="wg3", bufs=3) as wg3, \
             tc.tile_pool(name="xs", bufs=2) as xs, \
             tc.tile_pool(name="htp", bufs=1) as htp, \
             tc.tile_pool(name="gwp", bufs=2) as gwp, \
             tc.tile_pool(name="actp", bufs=3) as actp, \
             tc.tile_pool(name="stg", bufs=3) as stg, \
             tc.tile_pool(name="pg12", bufs=2, space="PSUM") as pg12, \
             tc.tile_pool(name="pg3", bufs=3, space="PSUM") as pg3:

            for s in range(ELOC):
                if s == 0:
                    # first i-tile's weights load before the bulky xg so the
                    # first matmul group starts as early as possible
                    w1t0 = wg12.tile([128, KD * 128], f32r, tag="w1t", name="w1t0_0")
                    nc.sync.dma_start(w1t0[:], w1[0, 0])
                    w3t0 = wg12.tile([128, KD * 128], f32r, tag="w3t", name="w3t0_0")
                    nc.sync.dma_start(w3t0[:], w3[0, 0])
                xg_s = xs.tile([128, KD, cap], f32r, tag="x", name=f"xg{s}")
                # split the load so the first token chunk (and with it the
                # first matmul group) is ready sooner
                for (c0, cw) in ctile_off:
                    nc.sync.dma_start(xg_s[:, :, c0:c0 + cw], xg[s, :, :, c0:c0 + cw])
                gw_s = gwp.tile([128, cap], f32, tag="gw", name=f"gw{s}")
                nc.sync.dma_start(gw_s[:], gw[s])

                ht = htp.tile([128, KI, cap], f32r, tag="ht", name=f"ht{s}")

                # GEMM1/2: hT[i, c] = silu(x @ W1^T) * (x @ W3^T), transposed
                for i in range(KI):
                    if s == 0 and i == 0:
                        w1t, w3t = w1t0, w3t0
                    else:
                        w1t = wg12.tile([128, KD * 128], f32r, tag="w1t", name=f"w1t{s}_{i}")
                        nc.sync.dma_start(w1t[:], w1[s, i])
                        w3t = wg12.tile([128, KD * 128], f32r, tag="w3t", name=f"w3t{s}_{i}")
                        nc.sync.dma_start(w3t[:], w3[s, i])
                    for (c0, cw) in ctile_off:
                        p1 = pg12.tile([128, cw], f32, tag="p1", name="p1")
                        p3 = pg12.tile([128, cw], f32, tag="p3", name="p3")
                        for ko in range(KD):
                            nc.tensor.matmul(
                                p1[:], w1t[:, ko * 128:(ko + 1) * 128],
                                xg_s[:, ko, c0:c0 + cw],
                                start=(ko == 0), stop=(ko == KD - 1))
                        for ko in range(KD):
                            nc.tensor.matmul(
                                p3[:], w3t[:, ko * 128:(ko + 1) * 128],
                                xg_s[:, ko, c0:c0 + cw],
                                start=(ko == 0), stop=(ko == KD - 1))
                        a1 = actp.tile([128, max(cchunks)], f32, tag="act", name="a1")
                        nc.scalar.activation(a1[:, :cw], p1[:], AF.Silu)
                        nc.vector.tensor_mul(ht[:, i, c0:c0 + cw], a1[:, :cw], p3[:])

                # GEMM3: yT[d, c] = (hT^T @ W2^T)^T * gate_weight
                for d in range(KD):
                    w2t = wg3.tile([128, KI * 128], f32r, tag="w2t", name=f"w2t{s}_{d}")
                    nc.sync.dma_start(w2t[:], w2[s, d])
                    for (c0, cw) in ctile_off:
                        py = pg3.tile([128, cw], f32, tag="py", name="py")
                        for io in range(KI):
                            nc.tensor.matmul(
                                py[:], w2t[:, io * 128:(io + 1) * 128],
                                ht[:, io, c0:c0 + cw],
                                start=(io == 0), stop=(io == KI - 1))
                        st = stg.tile([128, max(cchunks)], f32, tag="st", name="st")
                        nc.vector.tensor_mul(st[:, :cw], py[:], gw_s[:, c0:c0 + cw])
                        nc.sync.dma_start(yt[s, d, :, c0:c0 + cw], st[:, :cw])

        # -------- shared expert (inter-sharded), weights resident, fused ----
        with tc.tile_pool(name="wsr", bufs=1) as wsr, \
             tc.tile_pool(name="xss", bufs=2) as xss, \
             tc.tile_pool(name="hstp", bufs=2) as hstp, \
             tc.tile_pool(name="acts", bufs=3) as acts, \
             tc.tile_pool(name="stgs", bufs=3) as stgs, \
             tc.tile_pool(name="pg12s", bufs=2, space="PSUM") as pg12s, \
             tc.tile_pool(name="pg3s", bufs=3, space="PSUM") as pg3s:

            ws1r = wsr.tile([128, KS, KD * 128], f32r, tag="ws1r", name="ws1r")
            ws3r = wsr.tile([128, KS, KD * 128], f32r, tag="ws3r", name="ws3r")
            for i in range(KS):
                nc.sync.dma_start(ws1r[:, i], ws1[i])
                nc.sync.dma_start(ws3r[:, i], ws3[i])
            ws2r = wsr.tile([128, KD, KS * 128], f32r, tag="ws2r", name="ws2r")
            for d in range(KD):
                nc.sync.dma_start(ws2r[:, d], ws2[d])

            for ct in range(T // TCHUNK):
                xt_c = xss.tile([128, KD, TCHUNK], f32r, tag="xts", name=f"xt{ct}")
                nc.sync.dma_start(xt_c[:], xt[ct])
                hst = hstp.tile([128, KS, TCHUNK], f32r, tag="hst", name=f"hst{ct}")
                for i in range(KS):
                    p1 = pg12s.tile([128, TCHUNK], f32, tag="p1", name="p1")
                    p3 = pg12s.tile([128, TCHUNK], f32, tag="p3", name="p3")
                    for ko in range(KD):
                        nc.tensor.matmul(
                            p1[:], ws1r[:, i, ko * 128:(ko + 1) * 128],
                            xt_c[:, ko],
                            start=(ko == 0), stop=(ko == KD - 1))
                    for ko in range(KD):
                        nc.tensor.matmul(
                            p3[:], ws3r[:, i, ko * 128:(ko + 1) * 128],
                            xt_c[:, ko],
                            start=(ko == 0), stop=(ko == KD - 1))
                    a1 = acts.tile([128, TCHUNK], f32, tag="acts", name="a1")
                    nc.scalar.activation(a1[:], p1[:], AF.Silu)
                    nc.vector.tensor_mul(hst[:, i], a1[:], p3[:])

                for d in range(KD):
                    py = pg3s.tile([128, TCHUNK], f32, tag="py", name="py")
                    for io in range(KS):
                        nc.tensor.matmul(
                            py[:], ws2r[:, d, io * 128:(io + 1) * 128],
                            hst[:, io],
                            start=(io == 0), stop=(io == KS - 1))
                    st = stgs.tile([128, TCHUNK], f32, tag="stgs", name="st")
                    # alternate the PSUM->SBUF drain across two engines so it
                    # keeps up with the 3-matmul fill
                    if d % 2 == 0:
                        nc.scalar.activation(st[:], py[:], AF.Copy)
                    else:
                        nc.vector.tensor_copy(st[:], py[:])
                    nc.sync.dma_start(
                        zt[d, :, ct * TCHUNK:(ct + 1) * TCHUNK], st[:])

    nc.compile()
    return nc


def _get_nc(cap, cchunks):
    key = ("nc", cap, cchunks)
    if key not in _CACHE:
        _CACHE[key] = _build_nc(cap, cchunks)
    return _CACHE[key]


# ------------------------------------------------------------------ kernel --
def kernel(x, Wg, W1, W3, W2, Ws1, Ws3, Ws2):
    from concourse.bass_utils import run_bass_kernel_spmd

    x = np.asarray(x, dtype=np.float32)
    x2d = np.ascontiguousarray(x.reshape(T, D))
    Wg = np.asarray(Wg, dtype=np.float32)
    W1 = np.asarray(W1, dtype=np.float32)
    W3 = np.asarray(W3, dtype=np.float32)
    W2 = np.asarray(W2, dtype=np.float32)
    Ws1 = np.asarray(Ws1, dtype=np.float32)
    Ws3 = np.asarray(Ws3, dtype=np.float32)
    Ws2 = np.asarray(Ws2, dtype=np.float32)

    # ---- host routing + dispatch ----
    topi, weights = _route(x2d, Wg)
    flat_e = topi.ravel()
    flat_t = np.repeat(np.arange(T), TOPK)
    flat_w = weights.ravel()
    order = np.argsort(flat_e, kind="stable")
    se, st_, sw = flat_e[order], flat_t[order], flat_w[order]
    bounds = np.searchsorted(se, np.arange(E + 1))
    tok_of = [st_[bounds[e]:bounds[e + 1]] for e in range(E)]
    wt_of = [sw[bounds[e]:bounds[e + 1]] for e in range(E)]

    cap, cchunks = _pick_cap(max(len(t) for t in tok_of))

    # ---- build per-core input maps ----
    xt_full = x2d.T.reshape(KD, 128, T).transpose(1, 0, 2)  # [128, KD, T]
    xt_tiles = np.ascontiguousarray(
        np.stack([xt_full[:, :, i * TCHUNK:(i + 1) * TCHUNK]
                  for i in range(T // TCHUNK)]))

    in_maps = []
    for c in range(NCORES):
        exps = [ELOC * c + s for s in range(ELOC)]
        xg_c = np.stack([_tile_xT(x2d[tok_of[e]], cap) for e in exps])
        gw_c = np.zeros((ELOC, 128, cap), dtype=np.float32)
        for s, e in enumerate(exps):
            gw_c[s, :, :len(wt_of[e])] = wt_of[e][None, :]
        w1_c = np.stack([_tile_kxm(W1[e]) for e in exps])
        w3_c = np.stack([_tile_kxm(W3[e]) for e in exps])
        w2_c = np.stack([_tile_kxm(W2[e]) for e in exps])

        lo = c * SI_SHARD
        ws1_s = np.zeros((SI_PAD, D), dtype=np.float32)
        ws1_s[:SI_SHARD] = Ws1[lo:lo + SI_SHARD]
        ws3_s = np.zeros((SI_PAD, D), dtype=np.float32)
        ws3_s[:SI_SHARD] = Ws3[lo:lo + SI_SHARD]
        ws2_s = np.zeros((D, SI_PAD), dtype=np.float32)
        ws2_s[:, :SI_SHARD] = Ws2[:, lo:lo + SI_SHARD]

        in_maps.append({
            "xg": xg_c, "gw": gw_c, "w1": w1_c, "w3": w3_c, "w2": w2_c,
            "xt": xt_tiles,
            "ws1": _tile_kxm(ws1_s), "ws3": _tile_kxm(ws3_s),
            "ws2": _tile_kxm(ws2_s),
        })

    # ---- run on 8 cores ----
    shapes = {
        "xg": (ELOC, 128, KD, cap), "gw": (ELOC, 128, cap),
        "w1": (ELOC, KI, 128, KD * 128), "w3": (ELOC, KI, 128, KD * 128),
        "w2": (ELOC, KD, 128, KI * 128),
        "xt": (T // TCHUNK, 128, KD, TCHUNK),
        "ws1": (KS, 128, KD * 128), "ws3": (KS, 128, KD * 128),
        "ws2": (KD, 128, KS * 128),
    }
    for m in in_maps:
        for k, v in m.items():
            assert v.shape == shapes[k], (k, v.shape, shapes[k])
            assert v.dtype == np.float32, (k, v.dtype)

    nc = _get_nc(cap, cchunks)
    res = run_bass_kernel_spmd(nc, in_maps, core_ids=list(range(NCORES)))
    _CACHE["last_results"] = res

    # ---- combine on host ----
    # routed: yt[s, d, p, c] = w * Y[c, d*128+p]
    cat_tok = []
    cat_rows = []
    for c in range(NCORES):
        ytc = res.results[c]["yt"]  # [ELOC, KD, 128, cap]
        for s in range(ELOC):
            e = ELOC * c + s
            n = len(tok_of[e])
            rows = ytc[s].reshape(D, cap).T[:n]  # [n, D]
            cat_tok.append(tok_of[e])
            cat_rows.append(rows)
    cat_tok = np.concatenate(cat_tok)
    cat_rows = np.concatenate(cat_rows, axis=0)
    order = np.argsort(cat_tok, kind="stable")
    y = cat_rows[order].reshape(T, TOPK, D).sum(axis=1)

    # shared: sum partials, zt[d, p, t] = Z[t, d*128+p]
    z_acc = res.results[0]["zt"].astype(np.float32).copy()
    for c in range(1, NCORES):
        z_acc += res.results[c]["zt"]
    z = z_acc.reshape(D, T).T  # [T, D]

    return (y + z).reshape(1, T, D).astype(np.float32)


# revision 15
# speedup vs baseline: 1.1493x; 1.0073x over previous
"""MoE (group-limited top-k routing) Trainium2 kernel, expert-parallel on 8 cores.

Strategy:
  - Host (numpy): gate softmax + group-limited top-4 routing (control plane,
    ~0.06% of FLOPs), token dispatch (gather per expert) and final combine.
  - Device (8 NeuronCores, SPMD): core c owns experts 2c, 2c+1. Each expert's
    routed tokens (padded to an adaptive capacity) run the SwiGLU FFN in fp32r
    at full PE rate; the gate weight is fused into the down-proj epilogue.
    The shared expert is inter-dim sharded (2816/8=352, padded to 384 per
    core) and each core computes a partial z for all 2048 tokens; host sums
    the partials.
  - All device matmuls keep features on partitions and tokens on the moving
    free dim, so no transposes are needed anywhere on device. Host supplies
    every tensor pre-tiled in SBUF layout so all DMAs are contiguous.
  - Phase order: shared expert first (ws1/ws3 resident in SBUF, read once),
    then the two routed experts (weights streamed, read once).
"""

import numpy as np

# Model dims (hardcoded per problem spec nn_MoE_51616916963811)
D = 2048
INTER = 1408
E = 16
TOPK = 4
G = 4
TOPK_G = 2
T = 2048
SI = 2816           # shared inter dim
SI_SHARD = SI // 8  # 352
SI_PAD = 384        # padded to 3x128
ROUTE_SCALE = 1.0

NCORES = 8
ELOC = 2            # experts per core
TCHUNK = 512        # shared-expert token chunk
KD = D // 128       # 16 contraction chunks over D
KI = INTER // 128   # 11 tiles over INTER
KS = SI_PAD // 128  # 3 tiles over padded shared inter

CAP_MIN = 512       # capacity floor (expected count is exactly 512)

_CACHE = {}


def _pick_cap(max_count):
    """Round the max per-expert token count up to a multiple of 32.

    fp32r matmuls drop to 1/4 rate below a 256-wide moving dim, so chunks
    must stay >= 256: cap <= 512 is one chunk, else two halves."""
    cap = max(CAP_MIN, ((int(max_count) + 31) // 32) * 32)
    assert cap <= 1024
    if cap <= 512:
        return cap, (cap,)
    half = ((cap // 2) + 31) // 32 * 32
    return 2 * half, (half, half)


# ---------------------------------------------------------------- host gate --
def _route(x2d, Wg):
    """Replicates the reference gate in numpy float32.

    Returns topi [T, TOPK] int64 and weights [T, TOPK] float32."""
    logits = x2d.astype(np.float32) @ Wg.T.astype(np.float32)      # [T, E]
    m = logits.max(axis=-1, keepdims=True)
    ex = np.exp(logits - m)
    scores = ex / ex.sum(axis=-1, keepdims=True)                   # [T, E]
    sg = scores.reshape(T, G, E // G)
    gs = sg.max(axis=-1)                                           # [T, G]
    gidx = np.argsort(-gs, axis=1, kind="stable")[:, :TOPK_G]
    gmask = np.zeros((T, G), dtype=bool)
    np.put_along_axis(gmask, gidx, True, axis=1)
    masked = np.where(gmask[:, :, None], sg, -np.inf).reshape(T, E)
    topi = np.argsort(-masked, axis=1, kind="stable")[:, :TOPK]
    weights = np.take_along_axis(scores, topi, axis=1) * ROUTE_SCALE
    return topi, weights.astype(np.float32)


# ------------------------------------------------------------ host packing --
def _tile_kxm(w):
    """[R, C] weight -> lhsT tiles [R/128, 128(p), C/128 * 128] where
    tile[i, p, ko*128+m] = w[i*128+m, ko*128+p].  (w rows = output features,
    w cols = contraction dim.)"""
    R, C = w.shape
    ri, ci = R // 128, C // 128
    return np.ascontiguousarray(
        w.reshape(ri, 128, ci, 128).transpose(0, 3, 2, 1)
    ).reshape(ri, 128, ci * 128)


def _tile_xT(xrows, cap):
    """[n, D] activations -> [128(p), KD, cap] with xT[p, ko, c] = x[c, ko*128+p],
    zero-padded to cap tokens."""
    n = xrows.shape[0]
    out = np.zeros((128, KD, cap), dtype=np.float32)
    xt = xrows.T.reshape(KD, 128, n).transpose(1, 0, 2)  # [128, KD, n]
    out[:, :, :n] = xt
    return out


# ------------------------------------------------------------- bass kernel --
def _build_nc(cap, cchunks):
    import concourse.bass as bass
    import concourse.tile as tile
    from concourse import bacc, mybir

    f32 = mybir.dt.float32
    f32r = mybir.dt.float32r
    AF = mybir.ActivationFunctionType

    nc = bacc.Bacc("TRN2", target_bir_lowering=False, debug=False,
                   enable_asserts=False)

    # Inputs (per core). All pre-tiled on host; fp32r for matmul operands.
    xg = nc.dram_tensor("xg", [ELOC, 128, KD, cap], f32r, kind="ExternalInput").ap()
    gw = nc.dram_tensor("gw", [ELOC, 128, cap], f32, kind="ExternalInput").ap()
    w1 = nc.dram_tensor("w1", [ELOC, KI, 128, KD * 128], f32r, kind="ExternalInput").ap()
    w3 = nc.dram_tensor("w3", [ELOC, KI, 128, KD * 128], f32r, kind="ExternalInput").ap()
    w2 = nc.dram_tensor("w2", [ELOC, KD, 128, KI * 128], f32r, kind="ExternalInput").ap()
    xt = nc.dram_tensor("xt", [T // TCHUNK, 128, KD, TCHUNK], f32r, kind="ExternalInput").ap()
    ws1 = nc.dram_tensor("ws1", [KS, 128, KD * 128], f32r, kind="ExternalInput").ap()
    ws3 = nc.dram_tensor("ws3", [KS, 128, KD * 128], f32r, kind="ExternalInput").ap()
    ws2 = nc.dram_tensor("ws2", [KD, 128, KS * 128], f32r, kind="ExternalInput").ap()
    # Outputs
    yt = nc.dram_tensor("yt", [ELOC, KD, 128, cap], f32, kind="ExternalOutput").ap()
    zt = nc.dram_tensor("zt", [KD, 128, T], f32, kind="ExternalOutput").ap()

    ctile_off = []
    off = 0
    for w in cchunks:
        ctile_off.append((off, w))
        off += w
    NCT = T // TCHUNK

    with tile.TileContext(nc) as tc:
        # Long-lived pools: weight-stream slots (reused by the shared-expert
        # residents), token tiles, and PSUM (shared across phases so the
        # scheduler can weave shared-expert matmuls into routed stalls).
        wg12 = tc.alloc_tile_pool(name="wg12", bufs=3)
        xs = tc.alloc_tile_pool(name="xs", bufs=2)
        pg12 = tc.alloc_tile_pool(name="pg12", bufs=2, space="PSUM")
        pg3 = tc.alloc_tile_pool(name="pg3", bufs=3, space="PSUM")
        # routed-phase pools
        wg3 = tc.alloc_tile_pool(name="wg3", bufs=3)
        htp = tc.alloc_tile_pool(name="htp", bufs=1)
        gwp = tc.alloc_tile_pool(name="gwp", bufs=2)
        actp = tc.alloc_tile_pool(name="actp", bufs=3)
        stg = tc.alloc_tile_pool(name="stg", bufs=3)

        # ---------------- routed experts, weights streamed ----------------
        for s in range(ELOC):
            if s == 0:
                # first i-tile's weights load before the bulky xg so the
                # first matmul group starts as early as possible
                w1t0 = wg12.tile([128, KD * 128], f32r, tag="w1t", name="w1t0_0")
                nc.sync.dma_start(w1t0[:], w1[0, 0])
                w3t0 = wg12.tile([128, KD * 128], f32r, tag="w3t", name="w3t0_0")
                nc.sync.dma_start(w3t0[:], w3[0, 0])
            xg_s = xs.tile([128, KD, cap], f32r, tag="x", name=f"xg{s}")
            for (c0, cw) in ctile_off:
                nc.sync.dma_start(xg_s[:, :, c0:c0 + cw], xg[s, :, :, c0:c0 + cw])
            gw_s = gwp.tile([128, cap], f32, tag="gw", name=f"gw{s}")
            nc.sync.dma_start(gw_s[:], gw[s])

            ht = htp.tile([128, KI, cap], f32r, tag="ht", name=f"ht{s}")

            # GEMM1/2: hT[i, c] = silu(x @ W1^T) * (x @ W3^T), transposed
            for i in range(KI):
                if s == 0 and i == 0:
                    w1t, w3t = w1t0, w3t0
                else:
                    w1t = wg12.tile([128, KD * 128], f32r, tag="w1t", name=f"w1t{s}_{i}")
                    nc.sync.dma_start(w1t[:], w1[s, i])
                    w3t = wg12.tile([128, KD * 128], f32r, tag="w3t", name=f"w3t{s}_{i}")
                    nc.sync.dma_start(w3t[:], w3[s, i])
                for (c0, cw) in ctile_off:
                    p1 = pg12.tile([128, cw], f32, tag="p1", name="p1")
                    p3 = pg12.tile([128, cw], f32, tag="p3", name="p3")
                    for ko in range(KD):
                        nc.tensor.matmul(
                            p1[:], w1t[:, ko * 128:(ko + 1) * 128],
                            xg_s[:, ko, c0:c0 + cw],
                            start=(ko == 0), stop=(ko == KD - 1))
                    for ko in range(KD):
                        nc.tensor.matmul(
                            p3[:], w3t[:, ko * 128:(ko + 1) * 128],
                            xg_s[:, ko, c0:c0 + cw],
                            start=(ko == 0), stop=(ko == KD - 1))
                    a1 = actp.tile([128, max(cchunks)], f32, tag="act", name="a1")
                    nc.scalar.activation(a1[:, :cw], p1[:], AF.Silu)
                    nc.vector.tensor_mul(ht[:, i, c0:c0 + cw], a1[:, :cw], p3[:])

            # after the last G12, the w1t/w3t slots start taking the shared
            # expert's resident weights (loads overlap G3(e1))
            if s == ELOC - 1:
                ws1r = [None] * KS
                ws3r = [None] * KS
                for i in range(KS):
                    ws1r[i] = wg12.tile([128, KD * 128], f32r, tag="w1t",
                                        name=f"ws1r{i}")
                    nc.sync.dma_start(ws1r[i][:], ws1[i])
                    ws3r[i] = wg12.tile([128, KD * 128], f32r, tag="w3t",
                                        name=f"ws3r{i}")
                    nc.sync.dma_start(ws3r[i][:], ws3[i])

            # GEMM3: yT[d, c] = (hT^T @ W2^T)^T * gate_weight
            for d in range(KD):
                w2t = wg3.tile([128, KI * 128], f32r, tag="w2t", name=f"w2t{s}_{d}")
                nc.sync.dma_start(w2t[:], w2[s, d])
                for (c0, cw) in ctile_off:
                    py = pg3.tile([128, cw], f32, tag="py", name="py")
                    for io in range(KI):
                        nc.tensor.matmul(
                            py[:], w2t[:, io * 128:(io + 1) * 128],
                            ht[:, io, c0:c0 + cw],
                            start=(io == 0), stop=(io == KI - 1))
                    st = stg.tile([128, max(cchunks)], f32, tag="st", name="st")
                    nc.vector.tensor_mul(st[:, :cw], py[:], gw_s[:, c0:c0 + cw])
                    nc.sync.dma_start(yt[s, d, :, c0:c0 + cw], st[:, :cw])

        # routed-only pools give way to the shared expert's tiles
        stg.release()
        actp.release()
        gwp.release()
        htp.release()
        wg3.release()

        ws2rp = tc.alloc_tile_pool(name="ws2rp", bufs=1)
        ws2r = ws2rp.tile([128, KD, KS * 128], f32r, tag="ws2r", name="ws2r")
        for d in range(KD):
            nc.sync.dma_start(ws2r[:, d], ws2[d])
        hstp = tc.alloc_tile_pool(name="hstp", bufs=2)
        acts = tc.alloc_tile_pool(name="acts", bufs=3)
        stgs = tc.alloc_tile_pool(name="stgs", bufs=3)

        # -------- shared expert (inter-sharded), weights resident, fused ----
        xt_tiles = {0: xs.tile([128, KD, cap], f32r, tag="x", name="xt0")}
        nc.sync.dma_start(xt_tiles[0][:, :, :TCHUNK], xt[0])
        for ct in range(NCT):
            if ct + 1 < NCT:
                xt_tiles[ct + 1] = xs.tile([128, KD, cap], f32r, tag="x",
                                           name=f"xt{ct + 1}")
                nc.sync.dma_start(xt_tiles[ct + 1][:, :, :TCHUNK], xt[ct + 1])
            xt_c = xt_tiles.pop(ct)
            hst = hstp.tile([128, KS, TCHUNK], f32r, tag="hst", name=f"hst{ct}")
            for i in range(KS):
                p1 = pg12.tile([128, TCHUNK], f32, tag="p1", name="p1")
                p3 = pg12.tile([128, TCHUNK], f32, tag="p3", name="p3")
                for ko in range(KD):
                    nc.tensor.matmul(
                        p1[:], ws1r[i][:, ko * 128:(ko + 1) * 128],
                        xt_c[:, ko, :TCHUNK],
                        start=(ko == 0), stop=(ko == KD - 1))
                for ko in range(KD):
                    nc.tensor.matmul(
                        p3[:], ws3r[i][:, ko * 128:(ko + 1) * 128],
                        xt_c[:, ko, :TCHUNK],
                        start=(ko == 0), stop=(ko == KD - 1))
                a1 = acts.tile([128, TCHUNK], f32, tag="acts", name="a1")
                nc.scalar.activation(a1[:], p1[:], AF.Silu)
                nc.vector.tensor_mul(hst[:, i], a1[:], p3[:])

            for d in range(KD):
                py = pg3.tile([128, TCHUNK], f32, tag="py", name="py")
                for io in range(KS):
                    nc.tensor.matmul(
                        py[:], ws2r[:, d, io * 128:(io + 1) * 128],
                        hst[:, io],
                        start=(io == 0), stop=(io == KS - 1))
                st = stgs.tile([128, TCHUNK], f32, tag="stgs", name="st")
                # alternate the PSUM->SBUF drain across two engines so it
                # keeps up with the 3-matmul fill
                if d % 2 == 0:
                    nc.scalar.activation(st[:], py[:], AF.Copy)
                else:
                    nc.vector.tensor_copy(st[:], py[:])
                nc.sync.dma_start(
                    zt[d, :, ct * TCHUNK:(ct + 1) * TCHUNK], st[:])

        stgs.release()
        acts.release()
        hstp.release()
        ws2rp.release()
        pg3.release()
        pg12.release()
        xs.release()
        wg12.release()

    nc.compile()
    return nc


def _get_nc(cap, cchunks):
    key = ("nc", cap, cchunks)
    if key not in _CACHE:
        _CACHE[key] = _build_nc(cap, cchunks)
    return _CACHE[key]


# ------------------------------------------------------------------ kernel --
def kernel(x, Wg, W1, W3, W2, Ws1, Ws3, Ws2):
    from concourse.bass_utils import run_bass_kernel_spmd

    x = np.asarray(x, dtype=np.float32)
    x2d = np.ascontiguousarray(x.reshape(T, D))
    Wg = np.asarray(Wg, dtype=np.float32)
    W1 = np.asarray(W1, dtype=np.float32)
    W3 = np.asarray(W3, dtype=np.float32)
    W2 = np.asarray(W2, dtype=np.float32)
    Ws1 = np.asarray(Ws1, dtype=np.float32)
    Ws3 = np.asarray(Ws3, dtype=np.float32)
    Ws2 = np.asarray(Ws2, dtype=np.float32)

    # ---- host routing + dispatch ----
    topi, weights = _route(x2d, Wg)
    flat_e = topi.ravel()
    flat_t = np.repeat(np.arange(T), TOPK)
    flat_w = weights.ravel()
    order = np.argsort(flat_e, kind="stable")
    se, st_, sw = flat_e[order], flat_t[order], flat_w[order]
    bounds = np.searchsorted(se, np.arange(E + 1))
    tok_of = [st_[bounds[e]:bounds[e + 1]] for e in range(E)]
    wt_of = [sw[bounds[e]:bounds[e + 1]] for e in range(E)]

    cap, cchunks = _pick_cap(max(len(t) for t in tok_of))

    # ---- build per-core input maps ----
    xt_full = x2d.T.reshape(KD, 128, T).transpose(1, 0, 2)  # [128, KD, T]
    xt_tiles = np.ascontiguousarray(
        np.stack([xt_full[:, :, i * TCHUNK:(i + 1) * TCHUNK]
                  for i in range(T // TCHUNK)]))

    in_maps = []
    for c in range(NCORES):
        exps = [ELOC * c + s for s in range(ELOC)]
        xg_c = np.stack([_tile_xT(x2d[tok_of[e]], cap) for e in exps])
        gw_c = np.zeros((ELOC, 128, cap), dtype=np.float32)
        for s, e in enumerate(exps):
            gw_c[s, :, :len(wt_of[e])] = wt_of[e][None, :]
        w1_c = np.stack([_tile_kxm(W1[e]) for e in exps])
        w3_c = np.stack([_tile_kxm(W3[e]) for e in exps])
        w2_c = np.stack([_tile_kxm(W2[e]) for e in exps])

        lo = c * SI_SHARD
        ws1_s = np.zeros((SI_PAD, D), dtype=np.float32)
        ws1_s[:SI_SHARD] = Ws1[lo:lo + SI_SHARD]
        ws3_s = np.zeros((SI_PAD, D), dtype=np.float32)
        ws3_s[:SI_SHARD] = Ws3[lo:lo + SI_SHARD]
        ws2_s = np.zeros((D, SI_PAD), dtype=np.float32)
        ws2_s[:, :SI_SHARD] = Ws2[:, lo:lo + SI_SHARD]

        in_maps.append({
            "xg": xg_c, "gw": gw_c, "w1": w1_c, "w3": w3_c, "w2": w2_c,
            "xt": xt_tiles,
            "ws1": _tile_kxm(ws1_s), "ws3": _tile_kxm(ws3_s),
            "ws2": _tile_kxm(ws2_s),
        })

    # ---- run on 8 cores ----
    shapes = {
        "xg": (ELOC, 128, KD, cap), "gw": (ELOC, 128, cap),
        "w1": (ELOC, KI, 128, KD * 128), "w3": (ELOC, KI, 128, KD * 128),
        "w2": (ELOC, KD, 128, KI * 128),
        "xt": (T // TCHUNK, 128, KD, TCHUNK),
        "ws1": (KS, 128, KD * 128), "ws3": (KS, 128, KD * 128),
        "ws2": (KD, 128, KS * 128),
    }
    for m in in_maps:
        for k, v in m.items():
            assert v.shape == shapes[k], (k, v.shape, shapes[k])
            assert v.dtype == np.float32, (k, v.dtype)

    nc = _get_nc(cap, cchunks)
    res = run_bass_kernel_spmd(nc, in_maps, core_ids=list(range(NCORES)))
    _CACHE["last_results"] = res

    # ---- combine on host ----
    # routed: yt[s, d, p, c] = w * Y[c, d*128+p]
    cat_tok = []
    cat_rows = []
    for c in range(NCORES):
        ytc = res.results[c]["yt"]  # [ELOC, KD, 128, cap]
        for s in range(ELOC):
            e = ELOC * c + s
            n = len(tok_of[e])
            rows = ytc[s].reshape(D, cap).T[:n]  # [n, D]
            cat_tok.append(tok_of[e])
            cat_rows.append(rows)
    cat_tok = np.concatenate(cat_tok)
    cat_rows = np.concatenate(cat_rows, axis=0)
    order = np.argsort(cat_tok, kind="stable")
    y = cat_rows[order].reshape(T, TOPK, D).sum(axis=1)

    # shared: sum partials, zt[d, p, t] = Z[t, d*128+p]
    z_acc = res.results[0]["zt"].astype(np.float32).copy()
    for c in range(1, NCORES):
        z_acc += res.results[c]["zt"]
    z = z_acc.reshape(D, T).T  # [T, D]

    return (y + z).reshape(1, T, D).astype(np.float32)


# revision 16
# speedup vs baseline: 1.1778x; 1.0248x over previous
"""MoE (group-limited top-k routing) Trainium2 kernel, expert-parallel on 8 cores.

Strategy:
  - Host (numpy): gate softmax + group-limited top-4 routing (control plane,
    ~0.06% of FLOPs), token dispatch (gather per expert) and final combine.
  - Device (8 NeuronCores, SPMD): core c owns experts 2c, 2c+1. Each expert's
    routed tokens (padded to an adaptive capacity) run the SwiGLU FFN in fp32r
    at full PE rate; the gate weight is fused into the down-proj epilogue.
    The shared expert is inter-dim sharded (2816/8=352, padded to 384 per
    core) and each core computes a partial z for all 2048 tokens; host sums
    the partials.
  - All device matmuls keep features on partitions and tokens on the moving
    free dim, so no transposes are needed anywhere on device. Host supplies
    every tensor pre-tiled in SBUF layout so all DMAs are contiguous.
  - Phase order: shared expert first (ws1/ws3 resident in SBUF, read once),
    then the two routed experts (weights streamed, read once).
"""

import numpy as np

# Model dims (hardcoded per problem spec nn_MoE_51616916963811)
D = 2048
INTER = 1408
E = 16
TOPK = 4
G = 4
TOPK_G = 2
T = 2048
SI = 2816           # shared inter dim
SI_SHARD = SI // 8  # 352
SI_PAD = 384        # padded to 3x128
ROUTE_SCALE = 1.0

NCORES = 8
ELOC = 2            # experts per core
TCHUNK = 512        # shared-expert token chunk
KD = D // 128       # 16 contraction chunks over D
KI = INTER // 128   # 11 tiles over INTER
KS = SI_PAD // 128  # 3 tiles over padded shared inter

CAP_MIN = 512       # capacity floor (expected count is exactly 512)

_CACHE = {}


def _pick_cap(max_count):
    """Round the max per-expert token count up to a multiple of 32.

    fp32r matmuls drop to 1/4 rate below a 256-wide moving dim, so chunks
    must stay >= 256: cap <= 512 is one chunk, else two halves."""
    cap = max(CAP_MIN, ((int(max_count) + 31) // 32) * 32)
    assert cap <= 1024
    if cap <= 512:
        return cap, (cap,)
    half = ((cap // 2) + 31) // 32 * 32
    return 2 * half, (half, half)


# ---------------------------------------------------------------- host gate --
def _route(x2d, Wg):
    """Replicates the reference gate in numpy float32.

    Returns topi [T, TOPK] int64 and weights [T, TOPK] float32."""
    logits = x2d.astype(np.float32) @ Wg.T.astype(np.float32)      # [T, E]
    m = logits.max(axis=-1, keepdims=True)
    ex = np.exp(logits - m)
    scores = ex / ex.sum(axis=-1, keepdims=True)                   # [T, E]
    sg = scores.reshape(T, G, E // G)
    gs = sg.max(axis=-1)                                           # [T, G]
    gidx = np.argsort(-gs, axis=1, kind="stable")[:, :TOPK_G]
    gmask = np.zeros((T, G), dtype=bool)
    np.put_along_axis(gmask, gidx, True, axis=1)
    masked = np.where(gmask[:, :, None], sg, -np.inf).reshape(T, E)
    topi = np.argsort(-masked, axis=1, kind="stable")[:, :TOPK]
    weights = np.take_along_axis(scores, topi, axis=1) * ROUTE_SCALE
    return topi, weights.astype(np.float32)


# ------------------------------------------------------------ host packing --
def _tile_kxm(w):
    """[R, C] weight -> lhsT tiles [R/128, 128(p), C/128 * 128] where
    tile[i, p, ko*128+m] = w[i*128+m, ko*128+p].  (w rows = output features,
    w cols = contraction dim.)"""
    R, C = w.shape
    ri, ci = R // 128, C // 128
    return np.ascontiguousarray(
        w.reshape(ri, 128, ci, 128).transpose(0, 3, 2, 1)
    ).reshape(ri, 128, ci * 128)


def _tile_xT(xrows, cap):
    """[n, D] activations -> [128(p), KD, cap] with xT[p, ko, c] = x[c, ko*128+p],
    zero-padded to cap tokens."""
    n = xrows.shape[0]
    out = np.zeros((128, KD, cap), dtype=np.float32)
    xt = xrows.T.reshape(KD, 128, n).transpose(1, 0, 2)  # [128, KD, n]
    out[:, :, :n] = xt
    return out


# ------------------------------------------------------------- bass kernel --
def _build_nc(cap, cchunks):
    import concourse.bass as bass
    import concourse.tile as tile
    from concourse import bacc, mybir

    f32 = mybir.dt.float32
    f32r = mybir.dt.float32r
    AF = mybir.ActivationFunctionType

    nc = bacc.Bacc("TRN2", target_bir_lowering=False, debug=False,
                   enable_asserts=False)

    # Inputs (per core). All pre-tiled on host; fp32r for matmul operands.
    xg = nc.dram_tensor("xg", [ELOC, 128, KD, cap], f32r, kind="ExternalInput").ap()
    gw = nc.dram_tensor("gw", [ELOC, 128, cap], f32, kind="ExternalInput").ap()
    w1 = nc.dram_tensor("w1", [ELOC, KI, 128, KD * 128], f32r, kind="ExternalInput").ap()
    w3 = nc.dram_tensor("w3", [ELOC, KI, 128, KD * 128], f32r, kind="ExternalInput").ap()
    w2 = nc.dram_tensor("w2", [ELOC, KD, 128, KI * 128], f32r, kind="ExternalInput").ap()
    xt = nc.dram_tensor("xt", [T // TCHUNK, 128, KD, TCHUNK], f32r, kind="ExternalInput").ap()
    ws1 = nc.dram_tensor("ws1", [KS, 128, KD * 128], f32r, kind="ExternalInput").ap()
    ws3 = nc.dram_tensor("ws3", [KS, 128, KD * 128], f32r, kind="ExternalInput").ap()
    ws2 = nc.dram_tensor("ws2", [KD, 128, KS * 128], f32r, kind="ExternalInput").ap()
    # Outputs
    yt = nc.dram_tensor("yt", [ELOC, KD, 128, cap], f32, kind="ExternalOutput").ap()
    zt = nc.dram_tensor("zt", [KD, 128, T], f32, kind="ExternalOutput").ap()

    ctile_off = []
    off = 0
    for w in cchunks:
        ctile_off.append((off, w))
        off += w
    NCT = T // TCHUNK

    with tile.TileContext(nc) as tc:
        # Long-lived pools: weight-stream slots (reused by the shared-expert
        # residents), token tiles, and PSUM (shared across phases so the
        # scheduler can weave shared-expert matmuls into routed stalls).
        wg12 = tc.alloc_tile_pool(name="wg12", bufs=3)
        xs = tc.alloc_tile_pool(name="xs", bufs=2)
        pg12 = tc.alloc_tile_pool(name="pg12", bufs=2, space="PSUM")
        pg3 = tc.alloc_tile_pool(name="pg3", bufs=3, space="PSUM")
        # routed-phase pools
        wg3 = tc.alloc_tile_pool(name="wg3", bufs=3)
        htp = tc.alloc_tile_pool(name="htp", bufs=1)
        gwp = tc.alloc_tile_pool(name="gwp", bufs=2)
        actp = tc.alloc_tile_pool(name="actp", bufs=3)
        stg = tc.alloc_tile_pool(name="stg", bufs=3)

        # ---------------- routed experts, weights streamed ----------------
        for s in range(ELOC):
            if s == 0:
                # startup order: first i-tile weights and the first token
                # chunk ahead of everything else
                w1t0 = wg12.tile([128, KD * 128], f32r, tag="w1t", name="w1t0_0")
                nc.sync.dma_start(w1t0[:], w1[0, 0])
                xg_s = xs.tile([128, KD, cap], f32r, tag="x", name="xg0")
                c0, cw = ctile_off[0]
                nc.sync.dma_start(xg_s[:, :, c0:c0 + cw], xg[0, :, :, c0:c0 + cw])
                w3t0 = wg12.tile([128, KD * 128], f32r, tag="w3t", name="w3t0_0")
                nc.sync.dma_start(w3t0[:], w3[0, 0])
                for (c0, cw) in ctile_off[1:]:
                    nc.sync.dma_start(xg_s[:, :, c0:c0 + cw], xg[0, :, :, c0:c0 + cw])
                gw_s = gwp.tile([128, cap], f32, tag="gw", name="gw0")
                nc.sync.dma_start(gw_s[:], gw[0])
            else:
                # xg/gw for later experts are DMA'd mid-G3 of the previous
                # expert (queue-order: behind that expert's critical w2 loads)
                xg_s, gw_s = xg_next, gw_next

            ht = htp.tile([128, KI, cap], f32r, tag="ht", name=f"ht{s}")

            # GEMM1/2: hT[i, c] = silu(x @ W1^T) * (x @ W3^T), transposed
            for i in range(KI):
                if s == 0 and i == 0:
                    w1t, w3t = w1t0, w3t0
                else:
                    w1t = wg12.tile([128, KD * 128], f32r, tag="w1t", name=f"w1t{s}_{i}")
                    nc.sync.dma_start(w1t[:], w1[s, i])
                    w3t = wg12.tile([128, KD * 128], f32r, tag="w3t", name=f"w3t{s}_{i}")
                    nc.sync.dma_start(w3t[:], w3[s, i])
                for (c0, cw) in ctile_off:
                    p1 = pg12.tile([128, cw], f32, tag="p1", name="p1")
                    p3 = pg12.tile([128, cw], f32, tag="p3", name="p3")
                    for ko in range(KD):
                        nc.tensor.matmul(
                            p1[:], w1t[:, ko * 128:(ko + 1) * 128],
                            xg_s[:, ko, c0:c0 + cw],
                            start=(ko == 0), stop=(ko == KD - 1))
                    for ko in range(KD):
                        nc.tensor.matmul(
                            p3[:], w3t[:, ko * 128:(ko + 1) * 128],
                            xg_s[:, ko, c0:c0 + cw],
                            start=(ko == 0), stop=(ko == KD - 1))
                    a1 = actp.tile([128, max(cchunks)], f32, tag="act", name="a1")
                    nc.scalar.activation(a1[:, :cw], p1[:], AF.Silu)
                    nc.vector.tensor_mul(ht[:, i, c0:c0 + cw], a1[:, :cw], p3[:])

            # GEMM3: yT[d, c] = (hT^T @ W2^T)^T * gate_weight
            for d in range(KD):
                w2t = wg3.tile([128, KI * 128], f32r, tag="w2t", name=f"w2t{s}_{d}")
                nc.sync.dma_start(w2t[:], w2[s, d])
                for (c0, cw) in ctile_off:
                    py = pg3.tile([128, cw], f32, tag="py", name="py")
                    for io in range(KI):
                        nc.tensor.matmul(
                            py[:], w2t[:, io * 128:(io + 1) * 128],
                            ht[:, io, c0:c0 + cw],
                            start=(io == 0), stop=(io == KI - 1))
                    st = stg.tile([128, max(cchunks)], f32, tag="st", name="st")
                    nc.vector.tensor_mul(st[:, :cw], py[:], gw_s[:, c0:c0 + cw])
                    nc.sync.dma_start(yt[s, d, :, c0:c0 + cw], st[:, :cw])
                if s == 0 and d == 9:
                    # next expert's tokens: enqueue behind this expert's w2
                    xg_next = xs.tile([128, KD, cap], f32r, tag="x", name="xg1")
                    for (c0, cw) in ctile_off:
                        nc.sync.dma_start(xg_next[:, :, c0:c0 + cw],
                                          xg[1, :, :, c0:c0 + cw])
                    gw_next = gwp.tile([128, cap], f32, tag="gw", name="gw1")
                    nc.sync.dma_start(gw_next[:], gw[1])
                if s == ELOC - 1 and d == 5:
                    # shared-expert residents reuse the freed w1t/w3t slots;
                    # enqueue them behind the first half of this G3's w2 loads
                    ws1r = [None] * KS
                    ws3r = [None] * KS
                    for i in range(KS):
                        ws1r[i] = wg12.tile([128, KD * 128], f32r, tag="w1t",
                                            name=f"ws1r{i}")
                        nc.sync.dma_start(ws1r[i][:], ws1[i])
                        ws3r[i] = wg12.tile([128, KD * 128], f32r, tag="w3t",
                                            name=f"ws3r{i}")
                        nc.sync.dma_start(ws3r[i][:], ws3[i])
                if s == ELOC - 1 and d == 11:
                    # first shared token chunk, ahead of the phase switch
                    xt_tiles = {0: xs.tile([128, KD, cap], f32r, tag="x",
                                           name="xt0")}
                    nc.sync.dma_start(xt_tiles[0][:, :, :TCHUNK], xt[0])

        # routed-only pools give way to the shared expert's tiles
        stg.release()
        actp.release()
        gwp.release()
        htp.release()
        wg3.release()

        ws2rp = tc.alloc_tile_pool(name="ws2rp", bufs=1)
        ws2r = ws2rp.tile([128, KD, KS * 128], f32r, tag="ws2r", name="ws2r")
        for d in range(KD):
            nc.sync.dma_start(ws2r[:, d], ws2[d])
        hstp = tc.alloc_tile_pool(name="hstp", bufs=2)
        acts = tc.alloc_tile_pool(name="acts", bufs=3)
        stgs = tc.alloc_tile_pool(name="stgs", bufs=3)

        # -------- shared expert (inter-sharded), weights resident, fused ----
        for ct in range(NCT):
            if ct + 1 < NCT:
                xt_tiles[ct + 1] = xs.tile([128, KD, cap], f32r, tag="x",
                                           name=f"xt{ct + 1}")
                nc.sync.dma_start(xt_tiles[ct + 1][:, :, :TCHUNK], xt[ct + 1])
            xt_c = xt_tiles.pop(ct)
            hst = hstp.tile([128, KS, TCHUNK], f32r, tag="hst", name=f"hst{ct}")
            for i in range(KS):
                p1 = pg12.tile([128, TCHUNK], f32, tag="p1", name="p1")
                p3 = pg12.tile([128, TCHUNK], f32, tag="p3", name="p3")
                for ko in range(KD):
                    nc.tensor.matmul(
                        p1[:], ws1r[i][:, ko * 128:(ko + 1) * 128],
                        xt_c[:, ko, :TCHUNK],
                        start=(ko == 0), stop=(ko == KD - 1))
                for ko in range(KD):
                    nc.tensor.matmul(
                        p3[:], ws3r[i][:, ko * 128:(ko + 1) * 128],
                        xt_c[:, ko, :TCHUNK],
                        start=(ko == 0), stop=(ko == KD - 1))
                a1 = acts.tile([128, TCHUNK], f32, tag="acts", name="a1")
                nc.scalar.activation(a1[:], p1[:], AF.Silu)
                nc.vector.tensor_mul(hst[:, i], a1[:], p3[:])

            for d in range(KD):
                py = pg3.tile([128, TCHUNK], f32, tag="py", name="py")
                for io in range(KS):
                    nc.tensor.matmul(
                        py[:], ws2r[:, d, io * 128:(io + 1) * 128],
                        hst[:, io],
                        start=(io == 0), stop=(io == KS - 1))
                st = stgs.tile([128, TCHUNK], f32, tag="stgs", name="st")
                # alternate the PSUM->SBUF drain across two engines so it
                # keeps up with the 3-matmul fill
                if d % 2 == 0:
                    nc.scalar.activation(st[:], py[:], AF.Copy)
                else:
                    nc.vector.tensor_copy(st[:], py[:])
                nc.sync.dma_start(
                    zt[d, :, ct * TCHUNK:(ct + 1) * TCHUNK], st[:])

        stgs.release()
        acts.release()
        hstp.release()
        ws2rp.release()
        pg3.release()
        pg12.release()
        xs.release()
        wg12.release()

    nc.compile()
    return nc


def _get_nc(cap, cchunks):
    key = ("nc", cap, cchunks)
    if key not in _CACHE:
        _CACHE[key] = _build_nc(cap, cchunks)
    return _CACHE[key]


# ------------------------------------------------------------------ kernel --
def kernel(x, Wg, W1, W3, W2, Ws1, Ws3, Ws2):
    from concourse.bass_utils import run_bass_kernel_spmd

    x = np.asarray(x, dtype=np.float32)
    x2d = np.ascontiguousarray(x.reshape(T, D))
    Wg = np.asarray(Wg, dtype=np.float32)
    W1 = np.asarray(W1, dtype=np.float32)
    W3 = np.asarray(W3, dtype=np.float32)
    W2 = np.asarray(W2, dtype=np.float32)
    Ws1 = np.asarray(Ws1, dtype=np.float32)
    Ws3 = np.asarray(Ws3, dtype=np.float32)
    Ws2 = np.asarray(Ws2, dtype=np.float32)

    # ---- host routing + dispatch ----
    topi, weights = _route(x2d, Wg)
    flat_e = topi.ravel()
    flat_t = np.repeat(np.arange(T), TOPK)
    flat_w = weights.ravel()
    order = np.argsort(flat_e, kind="stable")
    se, st_, sw = flat_e[order], flat_t[order], flat_w[order]
    bounds = np.searchsorted(se, np.arange(E + 1))
    tok_of = [st_[bounds[e]:bounds[e + 1]] for e in range(E)]
    wt_of = [sw[bounds[e]:bounds[e + 1]] for e in range(E)]

    cap, cchunks = _pick_cap(max(len(t) for t in tok_of))

    # ---- build per-core input maps ----
    xt_full = x2d.T.reshape(KD, 128, T).transpose(1, 0, 2)  # [128, KD, T]
    xt_tiles = np.ascontiguousarray(
        np.stack([xt_full[:, :, i * TCHUNK:(i + 1) * TCHUNK]
                  for i in range(T // TCHUNK)]))

    in_maps = []
    for c in range(NCORES):
        exps = [ELOC * c + s for s in range(ELOC)]
        xg_c = np.stack([_tile_xT(x2d[tok_of[e]], cap) for e in exps])
        gw_c = np.zeros((ELOC, 128, cap), dtype=np.float32)
        for s, e in enumerate(exps):
            gw_c[s, :, :len(wt_of[e])] = wt_of[e][None, :]
        w1_c = np.stack([_tile_kxm(W1[e]) for e in exps])
        w3_c = np.stack([_tile_kxm(W3[e]) for e in exps])
        w2_c = np.stack([_tile_kxm(W2[e]) for e in exps])

        lo = c * SI_SHARD
        ws1_s = np.zeros((SI_PAD, D), dtype=np.float32)
        ws1_s[:SI_SHARD] = Ws1[lo:lo + SI_SHARD]
        ws3_s = np.zeros((SI_PAD, D), dtype=np.float32)
        ws3_s[:SI_SHARD] = Ws3[lo:lo + SI_SHARD]
        ws2_s = np.zeros((D, SI_PAD), dtype=np.float32)
        ws2_s[:, :SI_SHARD] = Ws2[:, lo:lo + SI_SHARD]

        in_maps.append({
            "xg": xg_c, "gw": gw_c, "w1": w1_c, "w3": w3_c, "w2": w2_c,
            "xt": xt_tiles,
            "ws1": _tile_kxm(ws1_s), "ws3": _tile_kxm(ws3_s),
            "ws2": _tile_kxm(ws2_s),
        })

    # ---- run on 8 cores ----
    shapes = {
        "xg": (ELOC, 128, KD, cap), "gw": (ELOC, 128, cap),
        "w1": (ELOC, KI, 128, KD * 128), "w3": (ELOC, KI, 128, KD * 128),
        "w2": (ELOC, KD, 128, KI * 128),
        "xt": (T // TCHUNK, 128, KD, TCHUNK),
        "ws1": (KS, 128, KD * 128), "ws3": (KS, 128, KD * 128),
        "ws2": (KD, 128, KS * 128),
    }
    for m in in_maps:
        for k, v in m.items():
            assert v.shape == shapes[k], (k, v.shape, shapes[k])
            assert v.dtype == np.float32, (k, v.dtype)

    nc = _get_nc(cap, cchunks)
    res = run_bass_kernel_spmd(nc, in_maps, core_ids=list(range(NCORES)))
    _CACHE["last_results"] = res

    # ---- combine on host ----
    # routed: yt[s, d, p, c] = w * Y[c, d*128+p]
    cat_tok = []
    cat_rows = []
    for c in range(NCORES):
        ytc = res.results[c]["yt"]  # [ELOC, KD, 128, cap]
        for s in range(ELOC):
            e = ELOC * c + s
            n = len(tok_of[e])
            rows = ytc[s].reshape(D, cap).T[:n]  # [n, D]
            cat_tok.append(tok_of[e])
            cat_rows.append(rows)
    cat_tok = np.concatenate(cat_tok)
    cat_rows = np.concatenate(cat_rows, axis=0)
    order = np.argsort(cat_tok, kind="stable")
    y = cat_rows[order].reshape(T, TOPK, D).sum(axis=1)

    # shared: sum partials, zt[d, p, t] = Z[t, d*128+p]
    z_acc = res.results[0]["zt"].astype(np.float32).copy()
    for c in range(1, NCORES):
        z_acc += res.results[c]["zt"]
    z = z_acc.reshape(D, T).T  # [T, D]

    return (y + z).reshape(1, T, D).astype(np.float32)
